# revision 33
# baseline (speedup 1.0000x reference)
"""Trainium2 Bass kernel for nn_MemoryAdapterLayer (8-core SPMD).

reference:
    query = x @ Wq.T + bq                  # [B,S,DM]
    scores = query @ memory.T              # [B,S,M] (per batch)
    weights = softmax(scores, -1)
    attended = weights @ memory            # [B,S,DM]
    transformed = attended @ Wm.T + bm     # [B,S,DQ]
    return (x, transformed)

Sharding: 8 cores = (batch b = core//2) x (sequence half h = core%2).
Each core computes transformed for its [1024, :] slice of one batch.
x is passed through on the host.

v3: the graded metric is ONE cold pass (input DMAs + compute from a
cold PE p-state). TimelineSim (validated within 2% of the graded
baseline) showed the old kernel lost ~50us at the start: biases were
the LAST DMA issued, so the step1 bias-activation (and every
downstream op) stalled until all 14MB of loads drained; PE idled and
its p-state ramp kept resetting. Fixes:
  - DMA order/chunking by need time: bias first, then wq, x(blk0),
    memT in 4 chunks, x(blk1), memA in 4 chunks, wm.
  - a PE warmup matmul chain (dummy accumulate on a memset tile)
    ramps the p-state to full clock while the first loads stream.
  - cross-block interleave: block 1's step1 fills block 0's
    softmax-Z/mul latency; sums/bc matmuls are woven between
    attended chains so PE never waits on the Z reduction.
  - output staged in 4 chunks per block -> DMA tail overlaps compute.
  - single rotating PSUM pool (7 banks + 1 warmup bank).

16-bit matmuls lower to LDWEIGHTS+MATMUL pairs (LDW engine-time-free;
PE streams 512-col f16 MMs at ~213ns warm). Scores run f16 (memT f16,
QT f16), attended bf16 (exp needs bf16 range: fixed shift softmax, no
row max), step5 f16. Measured rel err ~2.7e-3 (gate 2e-2).

All DMAs go through SWDGE (gpsimd): this container's walrus rejects
HWDGE semaphore waits on PE instructions. split_overflow_waits() caps
per-instruction sync waits at 1 (S3_LW/CTRL_NO slot limits here).
"""
import sys

import numpy as np

for _p in ("/opt/trn_rl_repo",):
    if _p not in sys.path:
        sys.path.insert(0, _p)

import ml_dtypes
import concourse.bass as bass
import concourse.mybir as mybir
from concourse import tile
from concourse.bass_utils import run_bass_kernel_spmd

B, S, M = 4, 2048, 4096
DQ, DM = 1024, 512
N_CORES = 8
SL = S // 2          # per-core sequence rows
NBLK = 2             # s-blocks of 512 per core
SB = 512             # s-block width
QT_T, DT_T, MT_T = DQ // 128, DM // 128, M // 128  # 8, 4, 32
SHIFT = 80.0
WARM = 8             # warmup matmuls (ramp PE while first loads stream)

# stream tensor column offsets
MT0 = 12 + 2 * QT_T * 512            # memT base (after bias + wq/x0 pairs)
X1 = MT0 + MT_T * 512                # x block-1 base
WM0 = X1 + QT_T * 512                # wm base
STRM_C = WM0 + DT_T * QT_T * 128     # total stream columns


def wq_off(qt, dt):
    return 12 + (qt // 2) * 2048 + (qt % 2) * 512 + dt * 128


def x_off(blk, qt):
    if blk == 0:
        return 12 + (qt // 2) * 2048 + 1024 + (qt % 2) * 512
    return X1 + qt * 512


def wm_off(dt, qt):
    return WM0 + (dt * QT_T + qt) * 128

F32R = mybir.dt.float32r
F32 = mybir.dt.float32
F16 = mybir.dt.float16
BF16 = mybir.dt.bfloat16

_counter = [0]


def _split_overflow_waits(nc, limit=1):
    """Walrus here rejects >1 sync wait per instruction: hoist excess waits
    onto same-engine NOPs inserted directly before the instruction."""
    for bb in nc.main_func.blocks:
        insts = list(bb.instructions)
        out = []
        dirty = False
        for ins in insts:
            si = ins.sync_info
            waits = list(si.on_wait) if si is not None else []
            if len(waits) > limit:
                extra = waits[: len(waits) - limit]
                keep = waits[len(waits) - limit:]
                for w in extra:
                    _counter[0] += 1
                    nop = mybir.InstNoOp(
                        name=f"waitfix-{_counter[0]}",
                        engine=ins.engine,
                        sync_info=mybir.SyncInfo(on_wait=[w], on_update=[]),
                        bass_nofuse=True,
                    )
                    nc.register_instruction(nop, overwrite=True)
                    out.append(nop)
                ins.sync_info = mybir.SyncInfo(
                    on_wait=keep, on_update=list(si.on_update)
                )
                dirty = True
            out.append(ins)
        if dirty:
            bb.instructions = out


def build(repeats=1, warm=WARM, loop=None):
    """repeats: python-unrolled copies of the full pass (loads+compute).
    loop: instead wrap ONE copy in a tc.For_i hardware loop of `loop`
    iterations (small NEFF, used for repeat-differential timing)."""
    from contextlib import ExitStack

    nc = bass.Bass("TRN2", debug=False, num_devices=N_CORES)
    AF = mybir.ActivationFunctionType

    # single stream-ordered f16 input (chunk boundaries can then interleave
    # bias/wq/x so the first matmul's operands land in one early DMA):
    #   [ bias 12 | 4x( wq qt-pair 1024 | x0 qt-pair 1024 ) | memT 16384
    #     | x1 4096 | wm 4096 ]
    strm_d = nc.dram_tensor("strm", [128, STRM_C], F16, kind="ExternalInput").ap()
    memA_d = nc.dram_tensor("memA", [128, MT_T * 512], BF16, kind="ExternalInput").ap()
    outT_d = nc.dram_tensor("outT", [128, NBLK * QT_T * SB], F16, kind="ExternalOutput").ap()

    with tile.TileContext(nc) as tc:
        with ExitStack() as ctx:
            res = ctx.enter_context(tc.tile_pool(name="res", bufs=1))
            qtp = ctx.enter_context(tc.tile_pool(name="qtp", bufs=2))
            exp = ctx.enter_context(tc.tile_pool(name="expp", bufs=1))
            att = ctx.enter_context(tc.tile_pool(name="attp", bufs=2))
            bcp = ctx.enter_context(tc.tile_pool(name="bcp", bufs=2))
            zp = ctx.enter_context(tc.tile_pool(name="zp", bufs=2))
            stp = ctx.enter_context(tc.tile_pool(name="stp", bufs=4))
            psp = ctx.enter_context(tc.tile_pool(name="psp", bufs=7, space="PSUM"))
            wps = ctx.enter_context(tc.tile_pool(name="wps", bufs=1, space="PSUM"))

            # resident tensors
            strm = res.tile([128, STRM_C], F16)
            memA = res.tile([128, MT_T * 512], BF16)
            ones16 = res.tile([128, SB], F16)
            onesb = res.tile([128, 1], BF16)
            onesr = res.tile([1, 128], BF16)
            nshift = res.tile([128, 1], F32)
            nc.gpsimd.memset(ones16[:], 1.0)

            def emit_body():
                # ---- input DMAs: stream chunks in need order; sizes grow
                # with slack (first chunk = exactly the first chain's data)
                cuts = [0, 2060, 4108, 6156, MT0,
                        MT0 + 2048, MT0 + 4096, MT0 + 8192, X1]
                for c in range(3):
                    sl = slice(cuts[c], cuts[c + 1])
                    nc.gpsimd.dma_start(strm[:, sl], strm_d[:, sl])
                # small constants: needed from ~10us on; issued after the
                # head chunks so they don't delay the first DMA gens
                nc.gpsimd.memset(onesb[:], 1.0)
                nc.gpsimd.memset(onesr[:], 1.0)
                nc.gpsimd.memset(nshift[:], -SHIFT)
                for c in range(3, len(cuts) - 1):
                    sl = slice(cuts[c], cuts[c + 1])
                    nc.gpsimd.dma_start(strm[:, sl], strm_d[:, sl])
                for c in range(2):
                    sl = slice(c * 16 * 512, (c + 1) * 16 * 512)
                    nc.gpsimd.dma_start(memA[:, sl], memA_d[:, sl])
                nc.gpsimd.dma_start(strm[:, X1:WM0], strm_d[:, X1:WM0])
                nc.gpsimd.dma_start(strm[:, WM0:STRM_C], strm_d[:, WM0:STRM_C])

                # ---- PE warmup: ramp p-state while loads stream ----
                if warm:
                    wt = wps.tile([128, SB], F32, tag="warm")
                    for i in range(warm):
                        nc.tensor.matmul(
                            wt[:], ones16[:, 0:128], ones16[:],
                            start=(i == 0), stop=(i == warm - 1),
                        )

                QT = [[None] * DT_T for _ in range(NBLK)]
                EX = [None] * MT_T
                ACC = [None] * DT_T
                ATT = [[None] * DT_T for _ in range(NBLK)]
                ZAC = [None] * NBLK
                RC = [None] * NBLK
                BC = [None] * NBLK

                def step1(blk):
                    # qt-outer: the first 4 matmuls need only wq/x's first
                    # qt tile, so PE starts ~4us earlier on the cold pass.
                    pq = [psp.tile([128, SB], F32, tag="ps", name=f"pq{d}")
                          for d in range(DT_T)]
                    for qt in range(QT_T):
                        for dt in range(DT_T):
                            wo = wq_off(qt, dt)
                            xo = x_off(blk, qt)
                            nc.tensor.matmul(
                                pq[dt][:],
                                strm[:, wo:wo + 128],
                                strm[:, xo:xo + SB],
                                start=(qt == 0), stop=(qt == QT_T - 1),
                                skip_group_check=True,
                            )
                    for dt in range(DT_T):
                        q_t = qtp.tile([128, SB], F16, tag=f"qt{dt}")
                        nc.scalar.activation(q_t[:], pq[dt][:], AF.Identity,
                                             bias=strm[:, dt:dt + 1])
                        QT[blk][dt] = q_t

                def scores(blk):
                    zacc = zp.tile([128, SB], F32, tag="z")
                    for j in range(MT_T):
                        ss = psp.tile([128, SB], F32, tag="ps")
                        for dt in range(DT_T):
                            mo = MT0 + j * 512 + dt * 128
                            nc.tensor.matmul(
                                ss[:],
                                strm[:, mo:mo + 128],
                                QT[blk][dt][:],
                                start=(dt == 0), stop=(dt == DT_T - 1),
                            )
                        ex = exp.tile([128, SB], BF16, tag=f"ex{j}")
                        nc.scalar.activation(ex[:], ss[:], AF.Exp, bias=nshift[:])
                        EX[j] = ex
                        if j == 0:
                            nc.vector.tensor_copy(zacc[:], ex[:])
                        else:
                            nc.vector.tensor_add(zacc[:], zacc[:], ex[:])
                    ZAC[blk] = zacc

                def att_chain(blk, dt):
                    acc = psp.tile([128, SB], F32, tag="ps")
                    for j in range(MT_T):
                        nc.tensor.matmul(
                            acc[:],
                            memA[:, j * 512 + dt * 128: j * 512 + (dt + 1) * 128],
                            EX[j][:],
                            start=(j == 0), stop=(j == MT_T - 1),
                        )
                    ACC[dt] = acc

                def sums(blk):
                    # one f32->bf16 rounding of zacc (0.2% on Z) makes the
                    # column-sum matmul 1 cyc/row instead of f32's 4.
                    z16 = zp.tile([128, SB], BF16, tag="z16")
                    nc.vector.tensor_copy(z16[:], ZAC[blk][:])
                    sm = psp.tile([1, SB], F32, tag="ps")
                    nc.tensor.matmul(sm[:], onesb[:], z16[:],
                                     start=True, stop=True)
                    # bf16 rc: exponent range fits 1/Z ~ 5e-6 (f16 would
                    # flush to subnormals); 0.2% rounding on the 1/Z scale.
                    rc = bcp.tile([1, SB], BF16, tag="rc")
                    with nc.allow_low_precision(reason="1/Z scale; bf16 0.2% << 2e-2 gate"):
                        nc.vector.reciprocal(rc[:], sm[:])
                    RC[blk] = rc

                def bcast(blk):
                    bc_ps = psp.tile([128, SB], F32, tag="ps")
                    nc.tensor.matmul(bc_ps[:], onesr[:], RC[blk][:],
                                     start=True, stop=True)
                    bc = bcp.tile([128, SB], F32, tag="bc")
                    nc.scalar.activation(bc[:], bc_ps[:], AF.Copy)
                    BC[blk] = bc

                def mul(blk, dt):
                    a_t = att.tile([128, SB], F16, tag=f"att{dt}")
                    nc.vector.tensor_mul(a_t[:], ACC[dt][:], BC[blk][:])
                    ATT[blk][dt] = a_t

                def step5(blk):
                    # out chunks: duals for qt0-5 (DMA gen overlaps the next
                    # chain), singles for qt6/7 (short critical tail).
                    stage = None
                    for qt in range(QT_T):
                        wide = 2 if qt < 6 else 1
                        if qt < 6 and qt % 2 == 0:
                            stage = stp.tile([128, 2 * SB], F16, tag="st")
                        elif qt >= 6:
                            stage = stp.tile([128, 2 * SB], F16, tag="st")
                        part = (qt % 2) * SB if qt < 6 else 0
                        p5 = psp.tile([128, SB], F32, tag="ps")
                        for dt in range(DT_T):
                            wo = wm_off(dt, qt)
                            nc.tensor.matmul(
                                p5[:],
                                strm[:, wo:wo + 128],
                                ATT[blk][dt][:],
                                start=(dt == 0), stop=(dt == DT_T - 1),
                            )
                        nc.scalar.activation(
                            stage[:, part:part + SB], p5[:],
                            AF.Identity, bias=strm[:, DT_T + qt:DT_T + qt + 1])
                        if (qt < 6 and qt % 2 == 1) or qt >= 6:
                            lo = qt - 1 if qt < 6 else qt
                            nc.gpsimd.dma_start(
                                outT_d[:, (blk * QT_T + lo) * SB:
                                       (blk * QT_T + lo + (2 if qt < 6 else 1)) * SB],
                                stage[:, 0:(2 if qt < 6 else 1) * SB],
                            )

                def block_tail(blk):
                    att_chain(blk, 0)
                    sums(blk)
                    att_chain(blk, 1)
                    bcast(blk)
                    att_chain(blk, 2)
                    mul(blk, 0)
                    mul(blk, 1)
                    att_chain(blk, 3)
                    mul(blk, 2)
                    mul(blk, 3)

                step1(0)
                scores(0)
                block_tail(0)
                step1(1)        # fills block 0's mul/bc latency on PE
                step5(0)
                scores(1)
                block_tail(1)
                step5(1)

            if loop is not None:
                with tc.For_i(0, loop):
                    emit_body()
            else:
                for _rep in range(repeats):
                    emit_body()

    _split_overflow_waits(nc)
    return nc


def pack_inputs(x, memory, Wq, bq, Wm, bm):
    """Host-side pre-swizzle into SBUF-shaped [128, F] per-core arrays."""
    f16 = np.float16
    # [128 q_in, qt, dt*128]: per-qt 512-col group is dt-major = stream layout
    wqT = Wq.reshape(DT_T, 128, QT_T, 128).transpose(3, 2, 0, 1).reshape(
        128, QT_T, DT_T * 128).astype(f16)
    wmT = Wm.reshape(QT_T, 128, DT_T, 128).transpose(3, 2, 0, 1).reshape(
        128, -1).astype(f16)
    bias = np.concatenate(
        [bq.reshape(DT_T, 128).T, bm.reshape(QT_T, 128).T], axis=1).astype(f16)
    in_maps = []
    for core in range(N_CORES):
        b, h = core // 2, core % 2
        xl = x[b, h * SL:(h + 1) * SL, :]                      # [1024 s, 1024 q]
        xT = xl.T.reshape(QT_T, 128, NBLK, SB).transpose(1, 2, 0, 3).astype(f16)
        mb = memory[b]                                          # [4096 m, 512 d]
        memT = mb.reshape(MT_T, 128, DT_T, 128).transpose(3, 0, 2, 1).reshape(
            128, -1).astype(f16)
        memA = np.ascontiguousarray(
            mb.reshape(MT_T, 128, DM).transpose(1, 0, 2).reshape(128, -1)
        ).astype(ml_dtypes.bfloat16)
        strm = np.empty((128, STRM_C), f16)
        strm[:, 0:12] = bias
        for qt in range(QT_T):
            strm[:, wq_off(qt, 0):wq_off(qt, 0) + 512] = wqT[:, qt]
            strm[:, x_off(0, qt):x_off(0, qt) + SB] = xT[:, 0, qt]
            strm[:, x_off(1, qt):x_off(1, qt) + SB] = xT[:, 1, qt]
        strm[:, MT0:X1] = memT
        strm[:, WM0:STRM_C] = wmT
        in_maps.append({"strm": np.ascontiguousarray(strm), "memA": memA})
    return in_maps


def unpack_output(results, x):
    transformed = np.empty((B, S, DQ), np.float32)
    for core in range(N_CORES):
        b, h = core // 2, core % 2
        o = results[core]["outT"].astype(np.float32)            # [128, 8192]
        t_loc = o.reshape(128, NBLK, QT_T, SB).transpose(1, 3, 2, 0).reshape(SL, DQ)
        transformed[b, h * SL:(h + 1) * SL, :] = t_loc
    return transformed


_NC_CACHE = {}


def kernel(x, memory, Wq, bq, Wm, bm):
    x = np.asarray(x, np.float32)
    memory = np.asarray(memory, np.float32)
    Wq = np.asarray(Wq, np.float32)
    bq = np.asarray(bq, np.float32)
    Wm = np.asarray(Wm, np.float32)
    bm = np.asarray(bm, np.float32)
    if "nc" not in _NC_CACHE:
        _NC_CACHE["nc"] = build()
    nc = _NC_CACHE["nc"]
    in_maps = pack_inputs(x, memory, Wq, bq, Wm, bm)
    res = run_bass_kernel_spmd(nc, in_maps, core_ids=list(range(N_CORES)))
    transformed = unpack_output(res.results, x)
    return (x, transformed)


# revision 36
# speedup vs baseline: 1.7675x; 1.7675x over previous
"""Trainium2 Bass kernel for nn_MemoryAdapterLayer (8-core SPMD).

reference:
    query = x @ Wq.T + bq                  # [B,S,DM]
    scores = query @ memory.T              # [B,S,M] (per batch)
    weights = softmax(scores, -1)
    attended = weights @ memory            # [B,S,DM]
    transformed = attended @ Wm.T + bm     # [B,S,DQ]
    return (x, transformed)

Sharding: 8 cores = (batch b = core//2) x (sequence half h = core%2).
Each core computes transformed for its [1024, :] slice of one batch.
x is passed through on the host.

v3: the graded metric is ONE cold pass (input DMAs + compute from a
cold PE p-state). TimelineSim (validated within 2% of the graded
baseline) showed the old kernel lost ~50us at the start: biases were
the LAST DMA issued, so the step1 bias-activation (and every
downstream op) stalled until all 14MB of loads drained; PE idled and
its p-state ramp kept resetting. Design (sim cold pass 148.5us vs
198us for the old kernel; PE-engine floor ~137us at f16 1 col/cycle):
  - all f16 inputs packed host-side into ONE stream-ordered DRAM
    tensor [bias | 4x(wq qt-pair | x0 qt-pair) | memT | x1 | wm], so
    chunk DMAs cut across logical tensors in exact need order; the
    first chunk carries precisely the first matmul chain's operands
    (first real MM at ~3.9us; SWDGE gen ~1us each, ~330GB/s stream).
  - an 8-matmul PE warmup chain (accumulate on a memset tile, result
    never read) ramps the p-state to 2.4GHz while the first chunk
    streams; real MMs then run at full clock from the start.
  - qt-outer step1 (4 concurrent PSUM accumulators) so the first 4
    matmuls need only the first wq/x tile pair.
  - cross-block interleave: block1's step1 fills block0's mul/bc
    latency; sums/bc matmuls are woven between attended chains.
  - softmax Z-chain: zacc f32 on DVE, one bf16 rounding before the
    column-sum matmul (1 cyc/row), reciprocal to bf16 (f16 would
    flush 1/Z~5e-6 to subnormals), bf16 broadcast matmul.
  - output staged per-qt, shipped as 3 dual chunks + 2 singles per
    block so the last DMA gen has minimal data left (tail ~4.7us:
    act+gen+copy+drain floor).
  - single rotating PSUM pool (7 banks, tag "ps") + 1 warmup bank.

16-bit matmuls lower to LDWEIGHTS+MATMUL pairs (LDW engine-time-free;
PE streams 512-col f16 MMs at ~213ns warm). Scores run f16 (memT f16,
QT f16), attended bf16 (exp needs bf16 range: fixed shift softmax, no
row max), step5 f16. Measured rel err ~3.1e-3 (gate 2e-2).

All DMAs go through SWDGE (gpsimd): this container's walrus rejects
HWDGE semaphore waits on PE instructions. split_overflow_waits() caps
per-instruction sync waits at 1 (S3_LW/CTRL_NO slot limits here).
"""
import sys

import numpy as np

for _p in ("/opt/trn_rl_repo",):
    if _p not in sys.path:
        sys.path.insert(0, _p)

import ml_dtypes
import concourse.bass as bass
import concourse.mybir as mybir
from concourse import tile
from concourse.bass_utils import run_bass_kernel_spmd

B, S, M = 4, 2048, 4096
DQ, DM = 1024, 512
N_CORES = 8
SL = S // 2          # per-core sequence rows
NBLK = 2             # s-blocks of 512 per core
SB = 512             # s-block width
QT_T, DT_T, MT_T = DQ // 128, DM // 128, M // 128  # 8, 4, 32
SHIFT = 80.0
WARM = 8             # warmup matmuls (ramp PE while first loads stream)

# stream tensor column offsets. bias padded to 32 cols so every section
# boundary is 64-byte aligned (HW DMA descriptors suffer on unaligned).
BIA = 32
MT0 = BIA + 2 * QT_T * 512           # memT base (after bias + wq/x0 pairs)
X1 = MT0 + MT_T * 512                # x block-1 base
WM0 = X1 + QT_T * 512                # wm base
STRM_C = WM0 + DT_T * QT_T * 128     # total stream columns


def wq_off(qt, dt):
    return BIA + (qt // 2) * 2048 + (qt % 2) * 512 + dt * 128


def x_off(blk, qt):
    if blk == 0:
        return BIA + (qt // 2) * 2048 + 1024 + (qt % 2) * 512
    return X1 + qt * 512


def wm_off(dt, qt):
    return WM0 + (dt * QT_T + qt) * 128

F32R = mybir.dt.float32r
F32 = mybir.dt.float32
F16 = mybir.dt.float16
BF16 = mybir.dt.bfloat16

_counter = [0]


def _split_overflow_waits(nc, limit=1):
    """Walrus here rejects >1 sync wait per instruction: hoist excess waits
    onto same-engine NOPs inserted directly before the instruction."""
    for bb in nc.main_func.blocks:
        insts = list(bb.instructions)
        out = []
        dirty = False
        for ins in insts:
            si = ins.sync_info
            waits = list(si.on_wait) if si is not None else []
            if len(waits) > limit:
                extra = waits[: len(waits) - limit]
                keep = waits[len(waits) - limit:]
                for w in extra:
                    _counter[0] += 1
                    nop = mybir.InstNoOp(
                        name=f"waitfix-{_counter[0]}",
                        engine=ins.engine,
                        sync_info=mybir.SyncInfo(on_wait=[w], on_update=[]),
                        bass_nofuse=True,
                    )
                    nc.register_instruction(nop, overwrite=True)
                    out.append(nop)
                ins.sync_info = mybir.SyncInfo(
                    on_wait=keep, on_update=list(si.on_update)
                )
                dirty = True
            out.append(ins)
        if dirty:
            bb.instructions = out


def build(repeats=1, warm=WARM, loop=None):
    """repeats: python-unrolled copies of the full pass (loads+compute).
    loop: instead wrap ONE copy in a tc.For_i hardware loop of `loop`
    iterations (small NEFF, used for repeat-differential timing)."""
    from contextlib import ExitStack

    nc = bass.Bass("TRN2", debug=False, num_devices=N_CORES)
    AF = mybir.ActivationFunctionType

    # single stream-ordered f16 input (chunk boundaries can then interleave
    # bias/wq/x so the first matmul's operands land in one early DMA):
    #   [ bias 12 | 4x( wq qt-pair 1024 | x0 qt-pair 1024 ) | memT 16384
    #     | x1 4096 | wm 4096 ]
    strm_d = nc.dram_tensor("strm", [128, STRM_C], F16, kind="ExternalInput").ap()
    memA_d = nc.dram_tensor("memA", [128, MT_T * 512], BF16, kind="ExternalInput").ap()
    outT_d = nc.dram_tensor("outT", [128, NBLK * QT_T * SB], F16, kind="ExternalOutput").ap()

    with tile.TileContext(nc) as tc:
        with ExitStack() as ctx:
            res = ctx.enter_context(tc.tile_pool(name="res", bufs=1))
            qtp = ctx.enter_context(tc.tile_pool(name="qtp", bufs=2))
            exp = ctx.enter_context(tc.tile_pool(name="expp", bufs=1))
            att = ctx.enter_context(tc.tile_pool(name="attp", bufs=2))
            bcp = ctx.enter_context(tc.tile_pool(name="bcp", bufs=2))
            zp = ctx.enter_context(tc.tile_pool(name="zp", bufs=2))
            stp = ctx.enter_context(tc.tile_pool(name="stp", bufs=4))
            psp = ctx.enter_context(tc.tile_pool(name="psp", bufs=7, space="PSUM"))
            wps = ctx.enter_context(tc.tile_pool(name="wps", bufs=1, space="PSUM"))

            # resident tensors
            strm = res.tile([128, STRM_C], F16)
            memA = res.tile([128, MT_T * 512], BF16)
            ones16 = res.tile([128, SB], F16)
            onesb = res.tile([128, 1], BF16)
            onesr = res.tile([1, 128], BF16)
            nshift = res.tile([128, 1], F32)
            nc.gpsimd.memset(ones16[:], 1.0)

            def emit_body():
                # ---- input DMAs: stream chunks in need order; sizes grow
                # with slack (first chunk = exactly the first chain's data)
                # bias has its own tiny DMA: its WAR hazard (step5 bias
                # reads at rep end) must not gate the wq/x chunk
                cuts = [BIA, 2048 + BIA, 4096 + BIA, 6144 + BIA, MT0,
                        MT0 + 2048, MT0 + 4096, MT0 + 8192, X1]
                for c in range(2):
                    sl = slice(cuts[c], cuts[c + 1])
                    nc.gpsimd.dma_start(strm[:, sl], strm_d[:, sl])
                # bias late (first use: step1's activation at ~11us) and
                # separate (its WAR hazard is step5's bias read at rep end)
                nc.gpsimd.dma_start(strm[:, 0:BIA], strm_d[:, 0:BIA])
                nc.gpsimd.dma_start(strm[:, cuts[2]:cuts[3]],
                                    strm_d[:, cuts[2]:cuts[3]])
                # small constants: needed from ~10us on; issued after the
                # head chunks so they don't delay the first DMA gens
                nc.gpsimd.memset(onesb[:], 1.0)
                nc.gpsimd.memset(onesr[:], 1.0)
                nc.gpsimd.memset(nshift[:], -SHIFT)
                for c in range(3, len(cuts) - 1):
                    sl = slice(cuts[c], cuts[c + 1])
                    nc.gpsimd.dma_start(strm[:, sl], strm_d[:, sl])
                for c in range(2):
                    sl = slice(c * 16 * 512, (c + 1) * 16 * 512)
                    nc.gpsimd.dma_start(memA[:, sl], memA_d[:, sl])
                nc.gpsimd.dma_start(strm[:, X1:WM0], strm_d[:, X1:WM0])
                nc.gpsimd.dma_start(strm[:, WM0:STRM_C], strm_d[:, WM0:STRM_C])

                # ---- PE warmup: ramp p-state while loads stream ----
                if warm:
                    wt = wps.tile([128, SB], F32, tag="warm")
                    for i in range(warm):
                        nc.tensor.matmul(
                            wt[:], ones16[:, 0:128], ones16[:],
                            start=(i == 0), stop=(i == warm - 1),
                        )

                QT = [[None] * DT_T for _ in range(NBLK)]
                EX = [None] * MT_T
                ACC = [None] * DT_T
                ATT = [[None] * DT_T for _ in range(NBLK)]
                ZAC = [None] * NBLK
                RC = [None] * NBLK
                BC = [None] * NBLK

                def step1(blk):
                    # qt-outer: the first 4 matmuls need only wq/x's first
                    # qt tile, so PE starts ~4us earlier on the cold pass.
                    pq = [psp.tile([128, SB], F32, tag="ps", name=f"pq{d}")
                          for d in range(DT_T)]
                    for qt in range(QT_T):
                        for dt in range(DT_T):
                            wo = wq_off(qt, dt)
                            xo = x_off(blk, qt)
                            nc.tensor.matmul(
                                pq[dt][:],
                                strm[:, wo:wo + 128],
                                strm[:, xo:xo + SB],
                                start=(qt == 0), stop=(qt == QT_T - 1),
                                skip_group_check=True,
                            )
                    for dt in range(DT_T):
                        q_t = qtp.tile([128, SB], F16, tag=f"qt{dt}")
                        nc.scalar.activation(q_t[:], pq[dt][:], AF.Identity,
                                             bias=strm[:, dt:dt + 1])
                        QT[blk][dt] = q_t

                def scores(blk):
                    zacc = zp.tile([128, SB], F32, tag="z")
                    for j in range(MT_T):
                        ss = psp.tile([128, SB], F32, tag="ps")
                        for dt in range(DT_T):
                            mo = MT0 + j * 512 + dt * 128
                            nc.tensor.matmul(
                                ss[:],
                                strm[:, mo:mo + 128],
                                QT[blk][dt][:],
                                start=(dt == 0), stop=(dt == DT_T - 1),
                            )
                        ex = exp.tile([128, SB], BF16, tag=f"ex{j}")
                        nc.scalar.activation(ex[:], ss[:], AF.Exp, bias=nshift[:])
                        EX[j] = ex
                        if j == 0:
                            nc.vector.tensor_copy(zacc[:], ex[:])
                        else:
                            nc.vector.tensor_add(zacc[:], zacc[:], ex[:])
                    ZAC[blk] = zacc

                def att_chain(blk, dt):
                    acc = psp.tile([128, SB], F32, tag="ps")
                    for j in range(MT_T):
                        nc.tensor.matmul(
                            acc[:],
                            memA[:, j * 512 + dt * 128: j * 512 + (dt + 1) * 128],
                            EX[j][:],
                            start=(j == 0), stop=(j == MT_T - 1),
                        )
                    ACC[dt] = acc

                def sums(blk):
                    # one f32->bf16 rounding of zacc (0.2% on Z) makes the
                    # column-sum matmul 1 cyc/row instead of f32's 4.
                    z16 = zp.tile([128, SB], BF16, tag="z16")
                    nc.vector.tensor_copy(z16[:], ZAC[blk][:])
                    sm = psp.tile([1, SB], F32, tag="ps")
                    nc.tensor.matmul(sm[:], onesb[:], z16[:],
                                     start=True, stop=True)
                    # bf16 rc: exponent range fits 1/Z ~ 5e-6 (f16 would
                    # flush to subnormals); 0.2% rounding on the 1/Z scale.
                    rc = bcp.tile([1, SB], BF16, tag="rc")
                    with nc.allow_low_precision(reason="1/Z scale; bf16 0.2% << 2e-2 gate"):
                        nc.vector.reciprocal(rc[:], sm[:])
                    RC[blk] = rc

                def bcast(blk):
                    bc_ps = psp.tile([128, SB], F32, tag="ps")
                    nc.tensor.matmul(bc_ps[:], onesr[:], RC[blk][:],
                                     start=True, stop=True)
                    bc = bcp.tile([128, SB], F32, tag="bc")
                    nc.scalar.activation(bc[:], bc_ps[:], AF.Copy)
                    BC[blk] = bc

                def mul(blk, dt):
                    a_t = att.tile([128, SB], F16, tag=f"att{dt}")
                    nc.vector.tensor_mul(a_t[:], ACC[dt][:], BC[blk][:])
                    ATT[blk][dt] = a_t

                def step5(blk):
                    # out chunks: duals for qt0-5 (DMA gen overlaps the next
                    # chain), singles for qt6/7 (short critical tail).
                    stage = None
                    for qt in range(QT_T):
                        wide = 2 if qt < 6 else 1
                        if qt < 6 and qt % 2 == 0:
                            stage = stp.tile([128, 2 * SB], F16, tag="st")
                        elif qt >= 6:
                            stage = stp.tile([128, 2 * SB], F16, tag="st")
                        part = (qt % 2) * SB if qt < 6 else 0
                        p5 = psp.tile([128, SB], F32, tag="ps")
                        for dt in range(DT_T):
                            wo = wm_off(dt, qt)
                            nc.tensor.matmul(
                                p5[:],
                                strm[:, wo:wo + 128],
                                ATT[blk][dt][:],
                                start=(dt == 0), stop=(dt == DT_T - 1),
                            )
                        nc.scalar.activation(
                            stage[:, part:part + SB], p5[:],
                            AF.Identity, bias=strm[:, DT_T + qt:DT_T + qt + 1])
                        if (qt < 6 and qt % 2 == 1) or qt >= 6:
                            lo = qt - 1 if qt < 6 else qt
                            nc.gpsimd.dma_start(
                                outT_d[:, (blk * QT_T + lo) * SB:
                                       (blk * QT_T + lo + (2 if qt < 6 else 1)) * SB],
                                stage[:, 0:(2 if qt < 6 else 1) * SB],
                            )

                def block_tail(blk):
                    att_chain(blk, 0)
                    sums(blk)
                    att_chain(blk, 1)
                    bcast(blk)
                    att_chain(blk, 2)
                    mul(blk, 0)
                    mul(blk, 1)
                    att_chain(blk, 3)
                    mul(blk, 2)
                    mul(blk, 3)

                step1(0)
                scores(0)
                block_tail(0)
                step1(1)        # fills block 0's mul/bc latency on PE
                step5(0)
                scores(1)
                block_tail(1)
                step5(1)

            if loop is not None:
                with tc.For_i(0, loop):
                    emit_body()
            else:
                for _rep in range(repeats):
                    emit_body()

    _split_overflow_waits(nc)
    return nc


def pack_inputs(x, memory, Wq, bq, Wm, bm):
    """Host-side pre-swizzle into SBUF-shaped [128, F] per-core arrays."""
    f16 = np.float16
    # [128 q_in, qt, dt*128]: per-qt 512-col group is dt-major = stream layout
    wqT = Wq.reshape(DT_T, 128, QT_T, 128).transpose(3, 2, 0, 1).reshape(
        128, QT_T, DT_T * 128).astype(f16)
    wmT = Wm.reshape(QT_T, 128, DT_T, 128).transpose(3, 2, 0, 1).reshape(
        128, -1).astype(f16)
    bias = np.concatenate(
        [bq.reshape(DT_T, 128).T, bm.reshape(QT_T, 128).T], axis=1).astype(f16)
    in_maps = []
    for core in range(N_CORES):
        b, h = core // 2, core % 2
        xl = x[b, h * SL:(h + 1) * SL, :]                      # [1024 s, 1024 q]
        xT = xl.T.reshape(QT_T, 128, NBLK, SB).transpose(1, 2, 0, 3).astype(f16)
        mb = memory[b]                                          # [4096 m, 512 d]
        memT = mb.reshape(MT_T, 128, DT_T, 128).transpose(3, 0, 2, 1).reshape(
            128, -1).astype(f16)
        memA = np.ascontiguousarray(
            mb.reshape(MT_T, 128, DM).transpose(1, 0, 2).reshape(128, -1)
        ).astype(ml_dtypes.bfloat16)
        strm = np.empty((128, STRM_C), f16)
        strm[:, 0:BIA] = 0.0
        strm[:, 0:12] = bias
        for qt in range(QT_T):
            strm[:, wq_off(qt, 0):wq_off(qt, 0) + 512] = wqT[:, qt]
            strm[:, x_off(0, qt):x_off(0, qt) + SB] = xT[:, 0, qt]
            strm[:, x_off(1, qt):x_off(1, qt) + SB] = xT[:, 1, qt]
        strm[:, MT0:X1] = memT
        strm[:, WM0:STRM_C] = wmT
        in_maps.append({"strm": np.ascontiguousarray(strm), "memA": memA})
    return in_maps


def unpack_output(results, x):
    transformed = np.empty((B, S, DQ), np.float32)
    for core in range(N_CORES):
        b, h = core // 2, core % 2
        o = results[core]["outT"].astype(np.float32)            # [128, 8192]
        t_loc = o.reshape(128, NBLK, QT_T, SB).transpose(1, 3, 2, 0).reshape(SL, DQ)
        transformed[b, h * SL:(h + 1) * SL, :] = t_loc
    return transformed


_NC_CACHE = {}


def kernel(x, memory, Wq, bq, Wm, bm):
    x = np.asarray(x, np.float32)
    memory = np.asarray(memory, np.float32)
    Wq = np.asarray(Wq, np.float32)
    bq = np.asarray(bq, np.float32)
    Wm = np.asarray(Wm, np.float32)
    bm = np.asarray(bm, np.float32)
    if "nc" not in _NC_CACHE:
        _NC_CACHE["nc"] = build()
    nc = _NC_CACHE["nc"]
    in_maps = pack_inputs(x, memory, Wq, bq, Wm, bm)
    res = run_bass_kernel_spmd(nc, in_maps, core_ids=list(range(N_CORES)))
    transformed = unpack_output(res.results, x)
    return (x, transformed)
